# revision 2
# baseline (speedup 1.0000x reference)
"""Mistral MoE layer (H=2048, F=8192, E=8, top-2) on 8 Trainium2 NeuronCores.

Strategy (expert parallelism per the sharding hint, with load-balancing
pairs):
  - Host computes the (tiny) gate: logits = x @ gate_w, top-2, softmax
    (0.004% of FLOPs; the expert FFNs dominate).
  - Experts are paired largest-load with smallest-load; each pair's two
    cores take ~half of BOTH experts' tokens ("all-to-all dispatch" done
    host-side for free).  Per-core capacity is (C1, C2) =
    (ceil(max_A/2), ceil(max_B/2)) ~= 1038 tokens instead of the max
    expert load (1058) or a 128-padded 1152 — the PE is the bottleneck
    (~96% busy at 78.6 TF/s), so capacity is execution time.
  - Core = two-segment SPMD SwiGLU FFN: segment-A tokens with expert-A
    weights, segment-B with expert-B.  Host scatter-adds the results.

Device kernel (per core):
  All matmul operands are bf16 (same 78.6 TF/s PE rate as f32r, half the
  HBM traffic; fp8 would be 2x but its ~4-5% quantization error fails the
  2e-2 gate).  Stage 1 computes yT = silu(w1.T x) * (w3.T x) in [F, C]
  layout.  Stage 2 keeps TOKENS ON THE MOVING DIM: out_T[H, C] accumulates
  w2_tile.T @ yT per F-group in PSUM, then into a bf16 SBUF accumulator —
  no DRAM bounce, and C needs no 128-multiple padding.  Gate weights are
  applied once at the end via an elementwise multiply with a
  partition-replicated gw tile (vector add + gpsimd multiply so the tail
  drains on two engines).

Fallback: routings too imbalanced for the pair template (C1+C2 > CAP)
run a single-expert-per-core kernel over multiple passes.
"""

import math
import os

import numpy as np
import ml_dtypes

import concourse.bass as bass
import concourse.mybir as mybir
import concourse.tile as tile
from concourse import bacc
from concourse.bass_utils import run_bass_kernel_spmd

P = 128
H = 2048
F = 8192
E = 8
TOP_K = 2

_kernel_cache: dict = {}

# Test-harness knobs (ignored in normal use): when TRACE is true, the SPMD
# run captures an NTFF profile and the BassKernelResults lands in LAST_RESULT.
TRACE = False
LAST_RESULT = None


def _chunks(C, off0=0, max_ch=512):
    """Split C into near-equal multiples of 2, each <= max_ch."""
    n = int(math.ceil(C / float(max_ch)))
    base = (C // n) // 2 * 2
    sizes = [base] * n
    rem = C - base * n
    i = 0
    while rem > 0:
        step = min(2, rem)
        sizes[i % n] += step
        rem -= step
        i += 1
    out = []
    off = off0
    for s in sizes:
        out.append((off, s))
        off += s
    assert off == off0 + C
    return out


def _stage1_ftile(nc, psum, spool, yt, xt_s, w_tiles, chunks, fi, CH, n_hh):
    """One f-tile of stage 1: ph/pu chains per chunk, silu, y-mult."""
    f32 = mybir.dt.float32
    for co, cs, seg in chunks:
        w1_t, w3_t = w_tiles[seg]
        csl = slice(co, co + cs)
        ph = psum.tile([P, CH], f32, tag="ph", bufs=2, name="ph")
        for hh in range(n_hh):
            nc.tensor.matmul(
                ph[:, :cs],
                w1_t[:, hh, :],
                xt_s[:, hh, csl],
                start=(hh == 0),
                stop=(hh == n_hh - 1),
            )
        pu = psum.tile([P, CH], f32, tag="pu", bufs=2, name="pu")
        for hh in range(n_hh):
            nc.tensor.matmul(
                pu[:, :cs],
                w3_t[:, hh, :],
                xt_s[:, hh, csl],
                start=(hh == 0),
                stop=(hh == n_hh - 1),
            )
        sl = spool.tile([P, CH], f32, tag="sl", name="sl")
        nc.scalar.activation(
            sl[:, :cs], ph[:, :cs], mybir.ActivationFunctionType.Silu
        )
        nc.vector.tensor_tensor(
            yt[:, fi, csl], sl[:, :cs], pu[:, :cs], mybir.AluOpType.mult
        )


def _stage2_group(nc, psum, opool, acc, yt, gw_s, out_r, w2_sel, chunks,
                  g, n_groups, n_ho, G, CH):
    """One group's stage 2: out_T partials, tokens on the moving dim."""
    f32 = mybir.dt.float32
    for co, cs, seg in chunks:
        w2_t = w2_sel[seg]
        csl = slice(co, co + cs)
        for ho in range(n_ho):
            po = psum.tile([P, CH], f32, tag="po", bufs=4, name="po")
            for fi in range(G):
                nc.tensor.matmul(
                    po[:, :cs],
                    w2_t[:, fi, bass.ts(ho, P)],
                    yt[:, fi, csl],
                    start=(fi == 0),
                    stop=(fi == G - 1),
                )
            asl = acc[:, ho, csl]
            if g == 0:
                nc.vector.tensor_scalar_mul(asl, po[:, :cs], 1.0)
            elif g < n_groups - 1:
                nc.vector.tensor_tensor(asl, asl, po[:, :cs], mybir.AluOpType.add)
            else:
                ot = opool.tile([P, CH], f32, tag="ot", name="ot")
                nc.vector.tensor_tensor(
                    ot[:, :cs], asl, po[:, :cs], mybir.AluOpType.add
                )
                nc.gpsimd.tensor_tensor(
                    ot[:, :cs], ot[:, :cs], gw_s[:, csl], mybir.AluOpType.mult
                )
                nc.sync.dma_start(out_r[:, ho, csl], ot[:, :cs])


def build_pair_kernel(C1, C2, H_=H, F_=F, G=8):
    """Two experts' SwiGLU FFN over C1+C2 tokens; returns finalized Bacc."""
    f32 = mybir.dt.float32
    bf16 = mybir.dt.bfloat16

    C = C1 + C2
    n_hh = H_ // P
    n_f = F_ // P
    n_ho = H_ // P
    n_groups = n_f // G
    # each chunk is (offset, size, segment); segment 0 -> expert A, 1 -> B
    chunks = [(o, s, 0) for o, s in _chunks(C1)] + [
        (o, s, 1) for o, s in _chunks(C2, off0=C1)
    ]
    CH = max(s for _, s, _ in chunks)

    nc = bacc.Bacc("TRN2", target_bir_lowering=False, debug=False)
    xt_d = nc.dram_tensor("xt", [H_, C], bf16, kind="ExternalInput")
    w1_da = nc.dram_tensor("w1pa", [F_, H_], bf16, kind="ExternalInput")
    w3_da = nc.dram_tensor("w3pa", [F_, H_], bf16, kind="ExternalInput")
    w2_da = nc.dram_tensor("w2ba", [F_, H_], bf16, kind="ExternalInput")
    w1_db = nc.dram_tensor("w1pb", [F_, H_], bf16, kind="ExternalInput")
    w3_db = nc.dram_tensor("w3pb", [F_, H_], bf16, kind="ExternalInput")
    w2_db = nc.dram_tensor("w2bb", [F_, H_], bf16, kind="ExternalInput")
    gw_d = nc.dram_tensor("gwr", [P, C], f32, kind="ExternalInput")
    out_d = nc.dram_tensor("out", [H_, C], f32, kind="ExternalOutput")

    xt_r = xt_d[:, :].rearrange("(hh hi) c -> hi hh c", hi=P)
    out_r = out_d[:, :].rearrange("(ho hi) c -> hi ho c", hi=P)

    with tile.TileContext(nc) as tc:
        with (
            tc.tile_pool(name="persist", bufs=1) as persist,
            tc.tile_pool(name="wpool", bufs=2) as wpool,
            tc.tile_pool(name="w2pool", bufs=1) as w2pool,
            tc.tile_pool(name="ypool", bufs=2) as ypool,
            tc.tile_pool(name="spool", bufs=2) as spool,
            tc.tile_pool(name="opool", bufs=3) as opool,
            tc.tile_pool(name="psum", bufs=1, space="PSUM") as psum,
        ):
            # f0's stationaries are issued before xt so the PE can start as
            # soon as xt lands; xt is fetched chunk-major in fine-grained
            # pieces round-robined over the three DMA-issuing engines.
            w1t0 = wpool.tile([P, n_hh, P], bf16, tag="w1ta", name="w1t0")
            nc.sync.dma_start(w1t0[:], w1_da[bass.ts(0, P), :])
            w3t0 = wpool.tile([P, n_hh, P], bf16, tag="w3ta", name="w3t0")
            nc.scalar.dma_start(w3t0[:], w3_da[bass.ts(0, P), :])
            w1t0b = wpool.tile([P, n_hh, P], bf16, tag="w1tb", name="w1t0b")
            nc.scalar.dma_start(w1t0b[:], w1_db[bass.ts(0, P), :])
            w3t0b = wpool.tile([P, n_hh, P], bf16, tag="w3tb", name="w3t0b")
            nc.sync.dma_start(w3t0b[:], w3_db[bass.ts(0, P), :])
            xt_s = persist.tile([P, n_hh, C], bf16, name="xt_s")
            engs = [nc.gpsimd, nc.sync, nc.scalar]
            k = 0
            for co, cs, _seg in chunks:
                for hh in range(n_hh):
                    engs[k % 3].dma_start(
                        xt_s[:, hh, co : co + cs], xt_r[:, hh, co : co + cs]
                    )
                    k += 1
            gw_s = persist.tile([P, C], f32, name="gw_s")
            nc.gpsimd.dma_start(gw_s[:], gw_d[:, :])
            acc = persist.tile([P, n_ho, C], bf16, name="acc")

            for g in range(n_groups):
                yt = ypool.tile([P, G, C], bf16, tag="yt", name="yt")
                w2_ta = w2pool.tile([P, G, H_], bf16, tag="w2ta", name="w2_ta")
                w2_tb = w2pool.tile([P, G, H_], bf16, tag="w2tb", name="w2_tb")

                # ---- stage 1
                for fi in range(G):
                    if fi == 2:
                        # w2 prefetch delayed past the startup xt burst
                        for fj in range(G):
                            nc.gpsimd.dma_start(
                                w2_ta[:, fj, :], w2_da[bass.ts(g * G + fj, P), :]
                            )
                            nc.gpsimd.dma_start(
                                w2_tb[:, fj, :], w2_db[bass.ts(g * G + fj, P), :]
                            )
                    f = g * G + fi
                    if f == 0:
                        w1a, w3a, w1b, w3b = w1t0, w3t0, w1t0b, w3t0b
                    else:
                        w1a = wpool.tile([P, n_hh, P], bf16, tag="w1ta", name="w1a")
                        nc.sync.dma_start(w1a[:], w1_da[bass.ts(f, P), :])
                        w3a = wpool.tile([P, n_hh, P], bf16, tag="w3ta", name="w3a")
                        nc.sync.dma_start(w3a[:], w3_da[bass.ts(f, P), :])
                        w1b = wpool.tile([P, n_hh, P], bf16, tag="w1tb", name="w1b")
                        nc.scalar.dma_start(w1b[:], w1_db[bass.ts(f, P), :])
                        w3b = wpool.tile([P, n_hh, P], bf16, tag="w3tb", name="w3b")
                        nc.scalar.dma_start(w3b[:], w3_db[bass.ts(f, P), :])
                    _stage1_ftile(
                        nc, psum, spool, yt, xt_s,
                        {0: (w1a, w3a), 1: (w1b, w3b)}, chunks, fi, CH, n_hh,
                    )

                # ---- stage 2
                _stage2_group(
                    nc, psum, opool, acc, yt, gw_s, out_r,
                    {0: w2_ta, 1: w2_tb}, chunks, g, n_groups, n_ho, G, CH,
                )
    nc.finalize()
    return nc


def build_expert_kernel(C, H_=H, F_=F, G=8):
    """Fallback: one expert's SwiGLU FFN over C tokens (multi-pass capable)."""
    f32 = mybir.dt.float32
    bf16 = mybir.dt.bfloat16

    n_hh = H_ // P
    n_f = F_ // P
    n_ho = H_ // P
    n_groups = n_f // G
    chunks = [(o, s, 0) for o, s in _chunks(C)]
    CH = max(s for _, s, _ in chunks)

    nc = bacc.Bacc("TRN2", target_bir_lowering=False, debug=False)
    xt_d = nc.dram_tensor("xt", [H_, C], bf16, kind="ExternalInput")
    w1_d = nc.dram_tensor("w1p", [F_, H_], bf16, kind="ExternalInput")
    w3_d = nc.dram_tensor("w3p", [F_, H_], bf16, kind="ExternalInput")
    w2_d = nc.dram_tensor("w2b", [F_, H_], bf16, kind="ExternalInput")
    gw_d = nc.dram_tensor("gwr", [P, C], f32, kind="ExternalInput")
    out_d = nc.dram_tensor("out", [H_, C], f32, kind="ExternalOutput")

    xt_r = xt_d[:, :].rearrange("(hh hi) c -> hi hh c", hi=P)
    out_r = out_d[:, :].rearrange("(ho hi) c -> hi ho c", hi=P)

    with tile.TileContext(nc) as tc:
        with (
            tc.tile_pool(name="persist", bufs=1) as persist,
            tc.tile_pool(name="wpool", bufs=2) as wpool,
            tc.tile_pool(name="w2pool", bufs=1) as w2pool,
            tc.tile_pool(name="ypool", bufs=2) as ypool,
            tc.tile_pool(name="spool", bufs=2) as spool,
            tc.tile_pool(name="opool", bufs=3) as opool,
            tc.tile_pool(name="psum", bufs=1, space="PSUM") as psum,
        ):
            w1_first = wpool.tile([P, n_hh, P], bf16, tag="w1t", name="w1_first")
            nc.sync.dma_start(w1_first[:], w1_d[bass.ts(0, P), :])
            w3_first = wpool.tile([P, n_hh, P], bf16, tag="w3t", name="w3_first")
            nc.scalar.dma_start(w3_first[:], w3_d[bass.ts(0, P), :])
            xt_s = persist.tile([P, n_hh, C], bf16, name="xt_s")
            engs = [nc.gpsimd, nc.sync, nc.scalar]
            k = 0
            for co, cs, _seg in chunks:
                for hh in range(n_hh):
                    engs[k % 3].dma_start(
                        xt_s[:, hh, co : co + cs], xt_r[:, hh, co : co + cs]
                    )
                    k += 1
            gw_s = persist.tile([P, C], f32, name="gw_s")
            nc.gpsimd.dma_start(gw_s[:], gw_d[:, :])
            acc = persist.tile([P, n_ho, C], bf16, name="acc")

            for g in range(n_groups):
                yt = ypool.tile([P, G, C], bf16, tag="yt", name="yt")
                w2_t = w2pool.tile([P, G, H_], bf16, tag="w2t", name="w2_t")
                for fi in range(G):
                    if fi == 2:
                        for fj in range(G):
                            nc.gpsimd.dma_start(
                                w2_t[:, fj, :], w2_d[bass.ts(g * G + fj, P), :]
                            )
                    f = g * G + fi
                    if f == 0:
                        w1_t, w3_t = w1_first, w3_first
                    else:
                        w1_t = wpool.tile([P, n_hh, P], bf16, tag="w1t", name="w1_t")
                        nc.sync.dma_start(w1_t[:], w1_d[bass.ts(f, P), :])
                        w3_t = wpool.tile([P, n_hh, P], bf16, tag="w3t", name="w3_t")
                        nc.scalar.dma_start(w3_t[:], w3_d[bass.ts(f, P), :])
                    _stage1_ftile(
                        nc, psum, spool, yt, xt_s,
                        {0: (w1_t, w3_t)}, chunks, fi, CH, n_hh,
                    )
                _stage2_group(
                    nc, psum, opool, acc, yt, gw_s, out_r,
                    {0: w2_t}, chunks, g, n_groups, n_ho, G, CH,
                )
    nc.finalize()
    return nc


def _route(x, gate_w):
    """Host gate: top-2 + softmax.  Returns (xs, idx per expert, weight per expert)."""
    xs = x.reshape(-1, x.shape[-1])
    logits = xs.astype(np.float32) @ gate_w.astype(np.float32)  # [T, E]
    # top-2 (ties broken by lower index, matching jax.lax.top_k)
    e1 = np.argmax(logits, axis=1)
    l1 = logits[np.arange(len(logits)), e1]
    masked = logits.copy()
    masked[np.arange(len(logits)), e1] = -np.inf
    e2 = np.argmax(masked, axis=1)
    l2 = masked[np.arange(len(logits)), e2]
    # softmax over the two logits
    w_hi = 1.0 / (1.0 + np.exp(l2 - l1))
    w_lo = 1.0 - w_hi
    idxs, gws = [], []
    for e in range(E):
        sel1 = e1 == e
        sel2 = e2 == e
        idx = np.nonzero(sel1 | sel2)[0]
        w = np.where(sel1[idx], w_hi[idx], w_lo[idx]).astype(np.float32)
        idxs.append(idx)
        gws.append(w)
    return xs, idxs, gws


def _pack_w13(w):
    """[H, F] f32 -> pre-tiled bf16 [(fo hi), (hh fj)] = [F, H] so each
    f-tile's stationary block [128, 16*128] is one contiguous DMA."""
    wb = w.astype(ml_dtypes.bfloat16)
    return np.ascontiguousarray(
        wb.reshape(H // P, P, F // P, P).transpose(2, 1, 0, 3).reshape(F, H)
    )


def _run(nc, in_maps):
    if TRACE:
        try:
            return run_bass_kernel_spmd(
                nc,
                in_maps,
                core_ids=list(range(E)),
                trace=True,
                trace_cores=list(range(E)),
            )
        except Exception as exc:
            import traceback

            print("TRACE FAILED:", exc)
            traceback.print_exc()
    return run_bass_kernel_spmd(nc, in_maps, core_ids=list(range(E)))


def _kernel_paired(xs, idxs, gws, w1, w3, w2, C1, C2):
    global LAST_RESULT
    bf = ml_dtypes.bfloat16
    loads = np.array([len(i) for i in idxs])
    order = np.argsort(-loads)
    pairs = [(int(order[i]), int(order[E - 1 - i])) for i in range(E // 2)]

    key = ("pair", C1, C2)
    if key not in _kernel_cache:
        _kernel_cache[key] = build_pair_kernel(C1, C2)
    nc = _kernel_cache[key]

    w_maps = {}
    for e in set(e for p in pairs for e in p):
        w_maps[e] = (
            _pack_w13(w1[e]),
            _pack_w13(w3[e]),
            np.ascontiguousarray(w2[e]).astype(bf),
        )

    in_maps = [None] * E
    core_meta = [None] * E
    for p, (ea, eb) in enumerate(pairs):
        a1 = (len(idxs[ea]) + 1) // 2
        b1 = (len(idxs[eb]) + 1) // 2
        splits = [
            ((idxs[ea][:a1], gws[ea][:a1]), (idxs[eb][:b1], gws[eb][:b1])),
            ((idxs[ea][a1:], gws[ea][a1:]), (idxs[eb][b1:], gws[eb][b1:])),
        ]
        for half in range(2):
            core = 2 * p + half
            (ia, ga), (ib, gb) = splits[half]
            na, nb = len(ia), len(ib)
            assert na <= C1 and nb <= C2
            xt = np.zeros((H, C1 + C2), bf)
            if na:
                xt[:, :na] = xs[ia].T.astype(bf)
            if nb:
                xt[:, C1 : C1 + nb] = xs[ib].T.astype(bf)
            gwpad = np.zeros(C1 + C2, np.float32)
            gwpad[:na] = ga
            gwpad[C1 : C1 + nb] = gb
            gwr = np.ascontiguousarray(
                np.broadcast_to(gwpad[None, :], (P, C1 + C2)), dtype=np.float32
            )
            in_maps[core] = {
                "xt": xt,
                "gwr": gwr,
                "w1pa": w_maps[ea][0],
                "w3pa": w_maps[ea][1],
                "w2ba": w_maps[ea][2],
                "w1pb": w_maps[eb][0],
                "w3pb": w_maps[eb][1],
                "w2bb": w_maps[eb][2],
            }
            core_meta[core] = (ia, ib, na, nb)

    res = _run(nc, in_maps)
    LAST_RESULT = res

    out_flat = np.zeros((xs.shape[0], H), np.float32)
    for core in range(E):
        ia, ib, na, nb = core_meta[core]
        o = res.results[core]["out"]
        if na:
            out_flat[ia] += o[:, :na].T
        if nb:
            out_flat[ib] += o[:, C1 : C1 + nb].T
    return out_flat


def _kernel_single(xs, idxs, gws, w1, w3, w2, C, n_pass):
    global LAST_RESULT
    bf = ml_dtypes.bfloat16

    key = ("single", C)
    if key not in _kernel_cache:
        _kernel_cache[key] = build_expert_kernel(C)
    nc = _kernel_cache[key]

    w_maps = [
        {
            "w1p": _pack_w13(w1[e]),
            "w3p": _pack_w13(w3[e]),
            "w2b": np.ascontiguousarray(w2[e]).astype(bf),
        }
        for e in range(E)
    ]

    out_flat = np.zeros((xs.shape[0], H), np.float32)
    for p in range(n_pass):
        in_maps = []
        p_idx = []
        for e in range(E):
            idx = idxs[e][p * C : (p + 1) * C]
            gw = gws[e][p * C : (p + 1) * C]
            n_e = len(idx)
            p_idx.append(idx)
            xt = np.zeros((H, C), bf)
            if n_e:
                xt[:, :n_e] = xs[idx].T.astype(bf)
            gwpad = np.zeros(C, np.float32)
            gwpad[:n_e] = gw
            gwr = np.ascontiguousarray(
                np.broadcast_to(gwpad[None, :], (P, C)), dtype=np.float32
            )
            in_maps.append({"xt": xt, "gwr": gwr, **w_maps[e]})
        res = _run(nc, in_maps)
        LAST_RESULT = res
        for e in range(E):
            n_e = len(p_idx[e])
            if n_e:
                out_flat[p_idx[e]] += res.results[e]["out"][:, :n_e].T
    return out_flat


def kernel(x, gate_w, w1, w3, w2):
    x = np.asarray(x)
    gate_w = np.asarray(gate_w)
    w1 = np.asarray(w1)
    w3 = np.asarray(w3)
    w2 = np.asarray(w2)

    xs, idxs, gws = _route(x, gate_w)
    loads = np.array([len(i) for i in idxs])
    max_load = int(loads.max())
    # SBUF budget bounds per-core capacity; more imbalanced routings fall
    # back to the single-expert kernel run over multiple token passes.
    CAP = int(os.environ.get("MOE_CAP", "1216"))

    order = np.argsort(-loads)
    pairs = [(int(order[i]), int(order[E - 1 - i])) for i in range(E // 2)]
    C1 = int(max(math.ceil(loads[a] / 2.0) for a, _ in pairs) + 1) // 2 * 2
    C2 = int(max(math.ceil(loads[b] / 2.0) for _, b in pairs) + 1) // 2 * 2
    C_single = max(260, int(math.ceil(max_load / 2.0)) * 2)

    if C1 + C2 <= min(CAP, C_single):
        out_flat = _kernel_paired(xs, idxs, gws, w1, w3, w2, C1, C2)
    else:
        C = min(CAP, C_single)
        n_pass = int(math.ceil(max_load / float(C)))
        out_flat = _kernel_single(xs, idxs, gws, w1, w3, w2, C, n_pass)
    return out_flat.reshape(x.shape)
